# revision 44
# baseline (speedup 1.0000x reference)
"""GraphSAGE fraud detector on 8 trn2 NeuronCores.

Strategy (dst-sharded graph parallel):
  - Nodes sharded across 8 cores (12500/core, padded to 12544 = 98*128).
  - Feature build: x/time on host-side layout, user/loc embedding gathers on
    device via dma_gather; time MLP via small matmuls.
  - Per layer: chunked AllGather of h (fp16, 2 chunks; the (half, core) row
    layout makes each chunk exactly two gather windows, so gathers start as
    soon as their chunk lands) -> per-core edge gather (dma_gather, 4 src
    windows of 25088 rows to fit int16 indices; the 4 windows map to the 4
    SWDGE queues so descriptor generation runs on all four Q7 core pairs
    concurrently) -> segment-sum via {0,1} one-hot matmuls (fp8 tables,
    host-precomputed, shared by both layers) accumulated in PSUM, then a
    per-dst 1/deg multiply on the vector engine -> dense W_l/W_r matmuls +
    bias + relu, interleaved into the stage loop per 512-col block.
  - Classifier: Wc matmul + sigmoid, fused into layer-2 dense blocks.

All data-dependent static structure (per-(group,bucket) chunk counts) is made
uniform across the 8 cores by padding each segment to the max across cores, so
one SPMD program serves all cores.
"""

import sys

sys.path.insert(0, "/opt/trn_rl_repo")

import contextlib
import ctypes
import types

import ml_dtypes
import numpy as np

import concourse.bacc as bacc
import concourse.bass as bass
import concourse.mybir as mybir
import concourse.tile as tile
from concourse.bass_utils import run_bass_kernel_spmd
from concourse.library_config import mlp
from concourse.masks import make_identity

F16 = mybir.dt.float16
F32 = mybir.dt.float32
F8 = mybir.dt.float8e4
I16 = mybir.dt.int16

N = 100000
E = 1600000
C = 8
NV = 12500          # valid nodes per core
NPC = 12544         # padded nodes per core (98*128)
NG = 98             # dst groups of 128 per core
SG = 7              # groups per stage
NSTAGES = 14
NB = 4              # src buckets
W = 2 * NPC         # src window (25088 < 32768, int16-safe)
NPAD = C * NPC      # 100352 = 4*W
HID = 128


def _ensure_axon_ntff_hook():
    """Register the axon NTFF profiling hook (trn_boot.py step 6 equivalent).

    The image's antenv stub lacks axon_hooks, so boot() degrades silently and
    run_bass_kernel_spmd(trace=True) skips profiling. Recreate the module and
    hook here so the reported exec time is the real neuron-profile HW time.
    """
    try:
        import antenv

        if "antenv.axon_hooks" not in sys.modules:
            mod = types.ModuleType("antenv.axon_hooks")
            holder = [None]
            mod.set_axon_ntff_profile_hook = lambda h: holder.__setitem__(0, h)
            mod.get_axon_ntff_profile_hook = lambda: holder[0]
            sys.modules["antenv.axon_hooks"] = mod
            antenv.axon_hooks = mod
        mod = sys.modules["antenv.axon_hooks"]
        if mod.get_axon_ntff_profile_hook() is not None:
            return True

        lib = ctypes.CDLL("/opt/axon/libaxon_pjrt.so")
        if not hasattr(lib, "axon_start_nrt_profile"):
            return False
        lib.axon_start_nrt_profile.argtypes = [
            ctypes.POINTER(ctypes.c_int64),
            ctypes.c_size_t,
        ]
        lib.axon_start_nrt_profile.restype = ctypes.c_int64
        lib.axon_stop_nrt_profile.argtypes = [ctypes.c_char_p]
        lib.axon_stop_nrt_profile.restype = ctypes.c_int64

        @contextlib.contextmanager
        def _hook(output_dir, device_ids):
            import jax

            jax.devices()
            if device_ids:
                ids = (ctypes.c_int64 * len(device_ids))(*device_ids)
                rc = lib.axon_start_nrt_profile(ids, len(device_ids))
            else:
                rc = lib.axon_start_nrt_profile(None, 0)
            if rc != 0:
                raise RuntimeError(f"axon_start_nrt_profile rc={rc}")
            try:
                yield
            finally:
                n = lib.axon_stop_nrt_profile(str(output_dir).encode())
                if n < 0:
                    raise RuntimeError(f"axon_stop_nrt_profile rc={n}")

        mod.set_axon_ntff_profile_hook(_hook)
        return True
    except Exception:
        return False


def _blocks():
    out = []
    w0 = 0
    while w0 < NPC:
        wd = min(512, NPC - w0)
        out.append((w0, wd))
        w0 += wd
    return out


def _time_chunks():
    # chunks of NPC//k, each a multiple of 128 (small: SBUF is tight)
    for tch in (256, 128):
        if NPC % tch == 0:
            return tch, NPC // tch
    raise ValueError


def _build_structure(edge_index):
    """Host-side edge preprocessing. Returns per-core arrays + uniform layout."""
    src = np.asarray(edge_index[0], dtype=np.int64)
    dst = np.asarray(edge_index[1], dtype=np.int64)
    deg = np.bincount(dst, minlength=N).astype(np.float32)
    invdeg = 1.0 / np.maximum(deg, 1.0)

    owner = dst // NV
    l = (dst - owner * NV).astype(np.int64)          # local dst id
    g = l // 128                                      # dst group
    dcol = (l - g * 128).astype(np.float32)           # col within group
    sown = src // NV
    sl = (src - sown * NV).astype(np.int64)           # local src id
    # chunked-AllGather layout: rows ordered (half, core, local-within-half)
    # so collective chunk h == bucket windows {2h, 2h+1}.
    HALF = NPC // 2                                   # 6272
    sh = sl // HALF                                   # src half (0/1)
    bkt = 2 * sh + sown // 4
    scol = ((sown % 4) * HALF + (sl - sh * HALF)).astype(np.int64)
    key = g * NB + bkt                                # (g, p) flat key
    counts = np.zeros((C, NG * NB), dtype=np.int64)
    per_core = []
    for c in range(C):
        m = owner == c
        kc = key[m]
        counts[c] = np.bincount(kc, minlength=NG * NB)
        per_core.append((kc, scol[m], dcol[m], invdeg[dst[m]]))

    Bseg = ((counts.max(axis=0) + 127) // 128) * 128  # uniform budgets [NG*NB]
    # equalize gather-call lengths across (stage, bucket) so the device
    # program needs a single num_idxs register value: pad the last group's
    # budget of each (s, p) so all stage-bucket sums match the global max.
    B2 = Bseg.reshape(NG, NB).copy()
    L_sp0 = np.array([[B2[s * SG:(s + 1) * SG, p].sum() for p in range(NB)]
                      for s in range(NSTAGES)])
    L_all = int(L_sp0.max())
    for s in range(NSTAGES):
        for p in range(NB):
            # spread the equalization pad across the stage's groups so no
            # single group's chunk count (and oh tile) balloons
            pad = (L_all - int(L_sp0[s, p])) // 128
            for j in range(pad):
                B2[s * SG + (j % SG), p] += 128
    Bseg = B2.reshape(NG * NB)
    # slot order: stage-major, then bucket, then group
    order_keys = []
    for s in range(NSTAGES):
        for p in range(NB):
            for gi in range(SG):
                order_keys.append((s * SG + gi) * NB + p)
    order_keys = np.array(order_keys)
    seg_len_ordered = Bseg[order_keys]
    seg_off_ordered = np.concatenate([[0], np.cumsum(seg_len_ordered)[:-1]])
    TOT = int(seg_len_ordered.sum())
    seg_off = np.zeros(NG * NB, dtype=np.int64)
    seg_off[order_keys] = seg_off_ordered

    core_arrays = []
    for c in range(C):
        kc, scol_c, dcol_c, inv_c = per_core[c]
        # secondary sort by src address within each segment: the DMA engines
        # then issue the random 256B reads in ascending order (HBM locality)
        o = np.lexsort((scol_c, kc))
        kc_s = kc[o]
        # position within each key group
        cnt = counts[c]
        starts = np.concatenate([[0], np.cumsum(cnt)[:-1]])
        pos = np.arange(len(kc_s)) - starts[kc_s]
        slot = seg_off[kc_s] + pos
        idx16 = np.zeros(TOT, dtype=np.int16)
        idx16[slot] = scol_c[o].astype(np.int16)
        idx_arr = np.tile(idx16.reshape(TOT // 16, 16).T, (8, 1))  # [128, TOT/16]
        # dense {0,1} one-hot table in fp8 (exact): partition-row p of column
        # block cc holds the one-hot row for gathered slot cc*128+p. The mean
        # (1/deg) is applied per-dst after aggregation on the vector engine.
        oh_arr = np.zeros((128, TOT), dtype=ml_dtypes.float8_e4m3)
        r = (slot % 128).astype(np.int64)
        ccol = (slot // 128).astype(np.int64)
        oh_arr[r, ccol * 128 + dcol_c[o].astype(np.int64)] = 1.0
        # per-dst 1/deg, broadcast across partitions [128, NPC]
        invl = np.ones(NPC, dtype=np.float16)
        invl[:NV] = invdeg[c * NV:(c + 1) * NV]
        inv_arr = np.tile(invl[None, :], (128, 1))
        core_arrays.append((np.ascontiguousarray(idx_arr),
                            np.ascontiguousarray(oh_arr),
                            np.ascontiguousarray(inv_arr)))
    return Bseg.reshape(NG, NB), seg_off.reshape(NG, NB), TOT, core_arrays


def _build_nc(Bseg, seg_off, TOT, use_cc=True):
    nc = bacc.Bacc("TRN2", num_devices=C, num_swdge_queues=4)
    TOT16 = TOT // 16

    # ---- parameters ----
    p_idx = nc.declare_dram_parameter("idx_all", [128, TOT16], I16, isOutput=False)
    p_oh = nc.declare_dram_parameter("oh_all", [128, TOT], F8, isOutput=False)
    p_inv = nc.declare_dram_parameter("inv_all", [128, NPC], F16, isOutput=False)
    p_x = nc.declare_dram_parameter("x16", [NPC, 64], F16, isOutput=False)
    p_t5 = nc.declare_dram_parameter("timeT5", [5, NPC], F16, isOutput=False)
    p_uidx = nc.declare_dram_parameter("uidx", [128, NPC // 16], I16, isOutput=False)
    p_lidx = nc.declare_dram_parameter("lidx", [128, NPC // 16], I16, isOutput=False)
    p_ut = nc.declare_dram_parameter("utab", [10000, 128], F16, isOutput=False)
    p_lt = nc.declare_dram_parameter("ltab", [1000, 128], F16, isOutput=False)
    p_wt5 = nc.declare_dram_parameter("wt5", [5, 16], F16, isOutput=False)
    p_w = {}
    for nm in ("w1l", "w1r", "w2l", "w2r"):
        p_w[nm] = nc.declare_dram_parameter(nm, [128, 128], F16, isOutput=False)
    p_b1 = nc.declare_dram_parameter("b1", [128, 1], F32, isOutput=False)
    p_b2 = nc.declare_dram_parameter("b2", [128, 1], F32, isOutput=False)
    p_wc = nc.declare_dram_parameter("wc", [128, 1], F16, isOutput=False)
    p_bc = nc.declare_dram_parameter("bc", [1, 1], F32, isOutput=False)
    p_out = nc.declare_dram_parameter("out", [1, NPC], F16, isOutput=True)

    cc_in = [nc.dram_tensor("cc_in0", [NPC, 128], F16),
             nc.dram_tensor("cc_in1", [NPC, 128], F16)]
    cc_out = [nc.dram_tensor("cc_out0", [NPAD, 128], F16, addr_space="Shared"),
              nc.dram_tensor("cc_out1", [NPAD, 128], F16, addr_space="Shared")]

    rg = [list(range(C))]

    # stage gather call layout
    L_sp = np.zeros((NSTAGES, NB), dtype=np.int64)
    O_sp = np.zeros((NSTAGES, NB), dtype=np.int64)
    for s in range(NSTAGES):
        for p in range(NB):
            L_sp[s, p] = Bseg[s * SG:(s + 1) * SG, p].sum()
            O_sp[s, p] = seg_off[s * SG, p]

    from contextlib import ExitStack

    with tile.TileContext(nc) as tc, ExitStack() as es:
        nc.gpsimd.load_library(mlp)
        _snap_cache = {}

        def snapv(v):
            if v not in _snap_cache:
                _snap_cache[v] = nc.gpsimd.snap(v)
            return _snap_cache[v]
        consts = es.enter_context(tc.tile_pool(name="consts", bufs=1))
        big = es.enter_context(tc.tile_pool(name="big", bufs=1))
        idxp = es.enter_context(tc.tile_pool(name="idxp", bufs=8))
        msgp = es.enter_context(tc.tile_pool(name="msgp", bufs=13))
        ohp = es.enter_context(tc.tile_pool(name="ohp", bufs=16))
        aggp = es.enter_context(tc.tile_pool(name="aggp", bufs=16))
        invp = es.enter_context(tc.tile_pool(name="invp", bufs=4))
        t5p = es.enter_context(tc.tile_pool(name="t5p", bufs=2))
        outp = es.enter_context(tc.tile_pool(name="outp", bufs=2))
        shipp = es.enter_context(tc.tile_pool(name="shipp", bufs=2))
        hobp = es.enter_context(tc.tile_pool(name="hobp", bufs=2))
        aggps = es.enter_context(tc.tile_pool(name="aggps", bufs=3, space="PSUM"))
        ps2p = es.enter_context(tc.tile_pool(name="ps2p", bufs=2, space="PSUM"))
        trp = es.enter_context(tc.tile_pool(name="trp", bufs=2, space="PSUM"))

        # ---- constants to SBUF ----
        wts = {nm: consts.tile_from(p_w[nm][:, :], name=nm) for nm in p_w}
        wt5_s = consts.tile_from(p_wt5[:, :])
        b1_s = consts.tile_from(p_b1[:, :])
        b2_s = consts.tile_from(p_b2[:, :])
        wc_s = consts.tile_from(p_wc[:, :])
        bc_s = consts.tile_from(p_bc[:, :])
        uidx_s = consts.tile_from(p_uidx[:, :])
        lidx_s = consts.tile_from(p_lidx[:, :])
        ident = consts.tile([128, 128], F16)
        make_identity(nc, ident[:, :])

        hT_cur = big.tile([128, NPC], F16, tag="hT0", name="hT0")

        # ---- phase 0: build h0 (node-major) ----
        h_nm = big.tile([128, NG * 128], F16, tag="hT1")
        h3 = h_nm[:, :].rearrange("p (g d) -> p g d", d=128)
        # user emb gather straight into h_nm (table cols 64:96 hold the emb);
        # split halves across SWDGE queues for concurrent desc-gen.
        HNPC = NPC // 2
        HG = NG // 2
        lg = big.tile([128, NG * 128], F16, tag="hT0")
        lg3 = lg[:, :].rearrange("p (g d) -> p g d", d=128)
        for half in range(2):
            g0, g1 = half * HG, (half + 1) * HG
            c0, c1 = half * (HNPC // 16), (half + 1) * (HNPC // 16)
            nc.gpsimd.dma_gather(h3[:, g0:g1, :], p_ut[:, :],
                                 uidx_s[:, c0:c1], HNPC, snapv(HNPC), 128,
                                 single_packet=False, queue_num=half)
            nc.gpsimd.dma_gather(lg3[:, g0:g1, :], p_lt[:, :],
                                 lidx_s[:, c0:c1], HNPC, snapv(HNPC), 128,
                                 single_packet=False, queue_num=2 + half)
        nc.vector.tensor_copy(h3[:, :, 96:112], lg3[:, :, 96:112])
        # x -> cols 0:64
        nc.sync.dma_start(
            out=h3[:, :, 0:64],
            in_=p_x[:, :].rearrange("(g p) d -> p g d", p=128),
        )
        # time mlp -> cols 112:128
        TCH, TNCH = _time_chunks()
        for t in range(TNCH):
            t5 = t5p.tile([5, TCH], F16, tag="t5")
            nc.sync.dma_start(out=t5[:, :], in_=p_t5[:, t * TCH:(t + 1) * TCH])
            for gi in range(TCH // 128):
                gg = t * (TCH // 128) + gi
                ps_t = trp.tile([128, 16], F32, tag="tr")
                nc.tensor.matmul(ps_t[:, :], t5[:, gi * 128:(gi + 1) * 128],
                                 wt5_s[:, :], start=True, stop=True)
                nc.scalar.activation(h3[:, gg, 112:128], ps_t[:, :],
                                     mybir.ActivationFunctionType.Copy)
        # hT0 via PE transpose
        for gg in range(NG):
            ps_tr = trp.tile([128, 128], F16, tag="tr")
            nc.tensor.transpose(ps_tr[:, :], h3[:, gg, :], ident[:, :])
            nc.vector.tensor_copy(hT_cur[:, gg * 128:(gg + 1) * 128], ps_tr[:, :])
        # ship h0 to collective input, one half at a time; each half feeds its
        # own AllGather chunk so layer-1 gathers on buckets {2h, 2h+1} can
        # start as soon as chunk h lands.
        HALF = NPC // 2

        def emit_cc(hv, h):
            if use_cc:
                nc.gpsimd.collective_compute(
                    "AllGather", mybir.AluOpType.bypass, replica_groups=rg,
                    ins=[cc_in[hv][h * HALF:(h + 1) * HALF, :]],
                    outs=[cc_out[hv][h * 2 * W:(h + 1) * 2 * W, :]],
                )
            else:
                nc.sync.dma_start(
                    out=cc_out[hv][h * 2 * W + (h * HALF):h * 2 * W + (h + 1) * HALF, :],
                    in_=cc_in[hv][h * HALF:(h + 1) * HALF, :])

        for h in range(2):
            g0, g1 = h * (NG // 2), (h + 1) * (NG // 2)
            nc.sync.dma_start(
                out=cc_in[0][g0 * 128:g1 * 128, :].rearrange(
                    "(g p) d -> p g d", p=128),
                in_=h3[:, g0:g1, :],
            )
            emit_cc(0, h)

        # ---- conv layers ----
        # dense/ship/classifier are interleaved into the stage loop: a dense
        # block fires as soon as the stages covering its groups are done, so
        # the tensor engine stays busy and only the very tail serializes.
        blocks = _blocks()
        for layer in range(2):
            wl = wts["w1l" if layer == 0 else "w2l"]
            wr = wts["w1r" if layer == 0 else "w2r"]
            bl = b1_s if layer == 0 else b2_s
            src_h = cc_out[layer]
            hT_in = hT_cur
            if layer == 0:
                hT_out = big.tile([128, NPC], F16, tag="hT1", name="hTo0")

            def emit_dense(w0, wd):
                # each dense chain owns a full PSUM bank and starts once, so
                # the start=True column-slice corruption cannot occur
                aggb = agg_tiles.pop(w0 // 512)
                ps2 = ps2p.tile([128, 512], F32, tag="ps2", name="ps2")
                nc.tensor.matmul(ps2[:, :wd], wl[:, :], aggb[:, :wd],
                                 start=True, stop=False)
                nc.tensor.matmul(ps2[:, :wd], wr[:, :], hT_in[:, w0:w0 + wd],
                                 start=False, stop=True)
                if layer == 0:
                    nc.scalar.activation(hT_out[:, w0:w0 + wd], ps2[:, :wd],
                                         mybir.ActivationFunctionType.Relu,
                                         bias=bl[:, :], scale=1.0)
                    # transpose to node-major and ship per group; fire each
                    # AllGather chunk as soon as its half is shipped
                    for gg in range(w0 // 128, (w0 + wd) // 128):
                        ps_tr = trp.tile([128, 128], F16, tag="tr")
                        nc.tensor.transpose(ps_tr[:, :],
                                            hT_out[:, gg * 128:(gg + 1) * 128],
                                            ident[:, :])
                        shp = shipp.tile([128, 128], F16, tag="ship")
                        nc.vector.tensor_copy(shp[:, :], ps_tr[:, :])
                        nc.sync.dma_start(
                            out=cc_in[1][gg * 128:(gg + 1) * 128, :],
                            in_=shp[:, :])
                        if gg == NG // 2 - 1:
                            emit_cc(1, 0)
                        elif gg == NG - 1:
                            emit_cc(1, 1)
                else:
                    hob = hobp.tile([128, wd], F16, tag="hob")
                    nc.scalar.activation(hob[:, :], ps2[:, :wd],
                                         mybir.ActivationFunctionType.Relu,
                                         bias=bl[:, :], scale=1.0)
                    ps3 = ps2p.tile([1, wd], F32, tag="ps3", name="ps3", bufs=1)
                    nc.tensor.matmul(ps3[:, :], wc_s[:, :], hob[:, :],
                                     start=True, stop=True)
                    ot = outp.tile([1, wd], F16, tag="ot")
                    nc.scalar.activation(ot[:, :], ps3[:, :],
                                         mybir.ActivationFunctionType.Sigmoid,
                                         bias=bc_s[0:1, 0:1], scale=1.0)
                    nc.sync.dma_start(out=p_out[0:1, w0:w0 + wd], in_=ot[:, :])

            next_block = 0
            agg_tiles = {}
            for s in range(NSTAGES):
                msgs = {}
                for p in range(NB):
                    L = int(L_sp[s, p])
                    if L == 0:
                        continue
                    O = int(O_sp[s, p])
                    it = idxp.tile([128, L // 16], I16, tag="idx")
                    nc.sync.dma_start(out=it[:, :],
                                      in_=p_idx[:, O // 16:(O + L) // 16])
                    mt = msgp.tile([128, (L // 128) * 128], F16, tag="msg")
                    nc.gpsimd.dma_gather(
                        mt[:, :].rearrange("p (b d) -> p b d", d=128),
                        src_h[p * W:(p + 1) * W, :],
                        it[:, :], L, snapv(L), 128, single_packet=False,
                        queue_num=p,
                    )
                    msgs[p] = mt
                for gi in range(SG):
                    gg = s * SG + gi
                    nchunks = int(Bseg[gg, :].sum()) // 128
                    ps = aggps.tile([128, 128], F32, tag="agg")
                    k = 0
                    for p in range(NB):
                        nb_gp = int(Bseg[gg, p]) // 128
                        if nb_gp == 0:
                            continue
                        mt = msgs[p]
                        mcol0 = (int(seg_off[gg, p]) - int(O_sp[s, p])) // 128
                        ccol0 = int(seg_off[gg, p]) // 128
                        oht = ohp.tile([128, nb_gp * 128], F8, tag="oh")
                        # oh loads ride the scalar engine's HWDGE queue so
                        # they don't sit behind the idx/ship queue on sync
                        nc.scalar.dma_start(
                            out=oht[:, :],
                            in_=p_oh[:, ccol0 * 128:(ccol0 + nb_gp) * 128])
                        for cch in range(nb_gp):
                            mc = mcol0 + cch
                            nc.tensor.matmul(
                                ps[:, :], mt[:, mc * 128:(mc + 1) * 128],
                                oht[:, cch * 128:(cch + 1) * 128],
                                start=(k == 0), stop=(k == nchunks - 1),
                            )
                            k += 1
                    # mean: multiply the summed PSUM by 1/deg per dst column,
                    # writing into a block-sized (4-group) agg tile so the
                    # dense matmul can consume 512 contiguous columns
                    invt = invp.tile([128, 128], F16, tag="inv")
                    nc.sync.dma_start(
                        out=invt[:, :],
                        in_=p_inv[:, gg * 128:(gg + 1) * 128])
                    if gg % 4 == 0:
                        agg_tiles[gg // 4] = aggp.tile([128, 512], F16,
                                                       tag="agg", name="aggb")
                    aggb = agg_tiles[gg // 4]
                    j = gg % 4
                    nc.vector.tensor_tensor(aggb[:, j * 128:(j + 1) * 128],
                                            ps[:, :], invt[:, :],
                                            op=mybir.AluOpType.mult)
                done_cols = (s + 1) * SG * 128
                while (next_block < len(blocks)
                       and blocks[next_block][0] + blocks[next_block][1]
                       <= done_cols):
                    emit_dense(*blocks[next_block])
                    next_block += 1
            while next_block < len(blocks):
                emit_dense(*blocks[next_block])
                next_block += 1
            if layer == 0:
                hT_cur = hT_out

    nc.compile()
    return nc


_CACHE = {}


def kernel(**inputs):
    x = np.asarray(inputs["x"], dtype=np.float32)
    edge_index = np.asarray(inputs["edge_index"])
    user_ids = np.asarray(inputs["user_ids"], dtype=np.int64)
    locations = np.asarray(inputs["locations"], dtype=np.int64)
    tf = np.asarray(inputs["time_features"], dtype=np.float32)

    Bseg, seg_off, TOT, core_arrays = _build_structure(edge_index)

    key = ("nc", TOT, tuple(Bseg.flatten().tolist()))
    if key not in _CACHE:
        _CACHE.clear()
        import os
        _CACHE[key] = _build_nc(Bseg, seg_off, TOT, use_cc=os.environ.get('NO_CC','0')!='1')
    nc = _CACHE[key]

    # shared (replicated) arrays
    ut = np.zeros((10000, 128), dtype=np.float16)
    ut[:, 64:96] = np.asarray(inputs["user_emb_table"], dtype=np.float32)
    lt = np.zeros((1000, 128), dtype=np.float16)
    lt[:, 96:112] = np.asarray(inputs["loc_emb_table"], dtype=np.float32)
    wt5 = np.concatenate(
        [np.asarray(inputs["W_time"], dtype=np.float32),
         np.asarray(inputs["b_time"], dtype=np.float32)[None, :]], axis=0
    ).astype(np.float16)
    shared = {
        "utab": ut, "ltab": lt, "wt5": wt5,
        "w1l": np.asarray(inputs["W1_l"], dtype=np.float16),
        "w1r": np.asarray(inputs["W1_r"], dtype=np.float16),
        "w2l": np.asarray(inputs["W2_l"], dtype=np.float16),
        "w2r": np.asarray(inputs["W2_r"], dtype=np.float16),
        "b1": np.asarray(inputs["b1"], dtype=np.float32).reshape(128, 1),
        "b2": np.asarray(inputs["b2"], dtype=np.float32).reshape(128, 1),
        "wc": np.asarray(inputs["Wc"], dtype=np.float16).reshape(128, 1),
        "bc": np.asarray(inputs["bc"], dtype=np.float32).reshape(1, 1),
    }

    in_maps = []
    for c in range(C):
        idx_arr, oh_arr, inv_arr = core_arrays[c]
        x16 = np.zeros((NPC, 64), dtype=np.float16)
        x16[:NV] = x[c * NV:(c + 1) * NV]
        t5 = np.ones((5, NPC), dtype=np.float16)
        t5[:4, :NV] = tf[c * NV:(c + 1) * NV].T
        t5[:4, NV:] = 0.0
        uid = np.zeros(NPC, dtype=np.int16)
        uid[:NV] = user_ids[c * NV:(c + 1) * NV]
        lid = np.zeros(NPC, dtype=np.int16)
        lid[:NV] = locations[c * NV:(c + 1) * NV]
        uidx = np.tile(uid.reshape(NPC // 16, 16).T, (8, 1))
        lidx = np.tile(lid.reshape(NPC // 16, 16).T, (8, 1))
        m = {
            "idx_all": idx_arr, "oh_all": oh_arr, "inv_all": inv_arr,
            "x16": x16, "timeT5": t5,
            "uidx": np.ascontiguousarray(uidx),
            "lidx": np.ascontiguousarray(lidx),
        }
        m.update(shared)
        in_maps.append(m)

    import os
    import time as _time

    traced = _ensure_axon_ntff_hook()
    trace_cores = list(range(C)) if os.environ.get("TRACE_ALL") == "1" else [0]
    _t0 = _time.perf_counter()
    res = run_bass_kernel_spmd(nc, in_maps, list(range(C)), trace=traced,
                               trace_cores=trace_cores)
    _t1 = _time.perf_counter()
    if getattr(res, "exec_time_ns", None):
        print(f"HW exec time: {res.exec_time_ns} ns")
    else:
        print(f"HW exec time: {int((_t1 - _t0) * 1e9)} ns (wall of spmd call, upper bound)")
    globals()["LAST_RESULT"] = res
    out = np.zeros((N, 1), dtype=np.float32)
    for c in range(C):
        o = np.asarray(res.results[c]["out"], dtype=np.float32).reshape(NPC)
        out[c * NV:(c + 1) * NV, 0] = o[:NV]
    return out
